# revision 3
# baseline (speedup 1.0000x reference)
"""Trainium2 Bass kernel for nn_Attention (B=4, C=256, L=2048, H=8 heads, D=64).

Sharding: (batch, q-window) across 8 NeuronCores — core j handles batch
j//2 and q columns [(j%2)*1024, (j%2)*1024+1024) for ALL 8 heads:
  - Q is projected for the core's 1024-column q-window (all heads),
  - K and V^T are projected over the full 2048 keys of the core's batch
    (2-way redundant across the two cores sharing a batch — cheap),
  - V^T is produced directly by a matmul with x as the stationary operand
    (no PE transposes), with an appended ones-column so the softmax
    denominator falls out of the PV matmul (M=65),
  - attention runs per head in the S^T (keys-on-partitions) layout,
  - normalization uses reciprocal_approx_fast (custom DVE op) + gpsimd
    partition broadcast,
  - w_out + bias are applied locally (all heads are core-local), so there
    are NO collectives anywhere.
Host reassembles the 8 [256, 1024] column slices into [B, C, L].

All matmul operands fp16 (fp32 PSUM accumulation).
"""

import sys

import numpy as np

sys.path.insert(0, "/opt/trn_rl_repo")

import concourse.bass as bass  # noqa: E402
import concourse.bacc as bacc  # noqa: E402
import concourse.tile as tile  # noqa: E402
import concourse.mybir as mybir  # noqa: E402
import concourse.bass_utils as bass_utils  # noqa: E402
from concourse.bass_interp import get_hw_module  # noqa: E402

B, C, L = 4, 256, 2048
H, D = 8, 64
NCORES = 8
QW = 1024                 # q-window per core
NB = 512                  # psum accumulation-group column width
F32 = mybir.dt.float32
F16 = mybir.dt.float16
AF = mybir.ActivationFunctionType

_CACHE = {}


def _build():
    nc = bacc.Bacc("TRN2", target_bir_lowering=False, debug=False,
                   num_devices=NCORES)

    xb_d = nc.dram_tensor("xb", [128, 2, L], F16, kind="ExternalInput")
    xq_d = nc.dram_tensor("xq", [128, 2, QW], F16, kind="ExternalInput")
    wq_d = nc.dram_tensor("wq", [128, 2, 512], F16, kind="ExternalInput")
    wk_d = nc.dram_tensor("wk", [128, 2, 512], F16, kind="ExternalInput")
    wv_d = nc.dram_tensor("wv", [128, 2, 512], F16, kind="ExternalInput")
    wo_d = nc.dram_tensor("wo", [128, 4, 256], F16, kind="ExternalInput")
    bias_d = nc.dram_tensor("bias2", [128, 2], F32, kind="ExternalInput")
    out = nc.dram_tensor("out", [2, 128, QW], F32, kind="ExternalOutput")

    with tile.TileContext(nc) as tc:
        with (
            tc.tile_pool(name="const", bufs=1) as cpool,
            tc.tile_pool(name="pt", bufs=3) as ptpool,
            tc.tile_pool(name="small", bufs=2) as spool,
            tc.tile_pool(name="psS", bufs=2, space="PSUM") as psS,
            tc.tile_pool(name="psO", bufs=2, space="PSUM") as psO,
        ):
            wq_sb = cpool.tile([128, 2, 512], F16, name="wq_sb")
            wk_sb = cpool.tile([128, 2, 512], F16, name="wk_sb")
            wv_sb = cpool.tile([128, 2, 512], F16, name="wv_sb")
            wo_sb = cpool.tile([128, 4, 256], F16, name="wo_sb")
            bias_sb = cpool.tile([128, 2], F32, name="bias_sb")
            xq_sb = cpool.tile([128, 2, QW], F16, name="xq_sb")
            xb_sb = cpool.tile([128, 2, L], F16, name="xb_sb")
            qd = cpool.tile([128, 4, QW], F16, name="qd")
            kd = cpool.tile([128, 4, L], F16, name="kd")
            vt = cpool.tile([128, 16, 8, 65], F16, name="vt")
            gh = cpool.tile([128, 4, QW], F16, name="gh")

            # weights + q-window x first (first projections need them)
            nc.sync.dma_start(wq_sb[:], wq_d[:])
            for cc in range(2):
                nc.sync.dma_start(xq_sb[:, cc, :], xq_d[:, cc, :])
            nc.sync.dma_start(wk_sb[:], wk_d[:])
            for half in range(2):
                for cc in range(2):
                    nc.sync.dma_start(
                        xb_sb[:, cc, half * 1024:(half + 1) * 1024],
                        xb_d[:, cc, half * 1024:(half + 1) * 1024])
            nc.sync.dma_start(wv_sb[:], wv_d[:])
            nc.sync.dma_start(wo_sb[:], wo_d[:])
            nc.sync.dma_start(bias_sb[:], bias_d[:])
            nc.vector.memset(vt[:, :, :, 64], 1.0)

            def emit_qproj(g):
                """Q projection for head pair g -> qd[:, g, :] (1024 q cols)."""
                psq = psS.tile([128, 1024], F32, name="psq", tag="psS")
                for c in range(2):
                    for cc in range(2):
                        nc.tensor.matmul(
                            psq[:, c * NB:(c + 1) * NB],
                            wq_sb[:, cc, g * 128:(g + 1) * 128],
                            xq_sb[:, cc, c * NB:(c + 1) * NB],
                            start=(cc == 0), stop=(cc == 1))
                nc.vector.tensor_copy(qd[:, g, :], psq[:])

            def emit_kproj(g, half):
                """K projection for head pair g, key half -> kd[:, g, half]."""
                psk = psS.tile([128, 1024], F32, name="psk", tag="psS")
                for c in range(2):
                    for cc in range(2):
                        nc.tensor.matmul(
                            psk[:, c * NB:(c + 1) * NB],
                            wk_sb[:, cc, g * 128:(g + 1) * 128],
                            xb_sb[:, cc, half * 1024 + c * NB:
                                  half * 1024 + (c + 1) * NB],
                            start=(cc == 0), stop=(cc == 1))
                nc.vector.tensor_copy(
                    kd[:, g, half * 1024:(half + 1) * 1024], psk[:])

            def emit_vt(lcp):
                """V^T for key chunks 2*lcp, 2*lcp+1 (x stationary, w_v moving)."""
                psv = psS.tile([128, 1024], F32, name="psv", tag="psS")
                for sub in range(2):
                    lc = 2 * lcp + sub
                    for cc in range(2):
                        nc.tensor.matmul(
                            psv[:, sub * NB:(sub + 1) * NB],
                            xb_sb[:, cc, lc * 128:(lc + 1) * 128],
                            wv_sb[:, cc, :],
                            start=(cc == 0), stop=(cc == 1))
                for sub in range(2):
                    nc.vector.tensor_copy(
                        vt[:, 2 * lcp + sub, :, 0:64],
                        psv[:, sub * NB:(sub + 1) * NB].rearrange(
                            "p (h d) -> p h d", h=8))

            emit_qproj(0)
            emit_kproj(0, 0)
            emit_kproj(0, 1)
            for lcp in range(8):
                emit_vt(lcp)

            for h in range(H):
                g, hp = h // 2, h % 2
                p0 = hp * 64
                pso = psO.tile([65, 1024], F32, name="pso", tag="pso")
                for kc in range(16):
                    pss = psS.tile([128, 1024], F32, name="pss", tag="psS")
                    for c in range(2):
                        nc.tensor.matmul(
                            pss[:, c * NB:(c + 1) * NB],
                            kd[p0:p0 + 64, g, kc * 128:(kc + 1) * 128],
                            qd[p0:p0 + 64, g, c * NB:(c + 1) * NB],
                            start=True, stop=True)
                    pt = ptpool.tile([128, 1024], F16, name="pt", tag="pt")
                    nc.scalar.activation(pt[:], pss[:], AF.Exp)
                    for c in range(2):
                        nc.tensor.matmul(
                            pso[:, c * NB:(c + 1) * NB],
                            vt[:, kc, h, :],
                            pt[:, c * NB:(c + 1) * NB],
                            start=(kc == 0), stop=(kc == 15))
                    if hp == 1 and g < 3 and kc == 8:
                        emit_qproj(g + 1)
                        emit_kproj(g + 1, 0)
                        emit_kproj(g + 1, 1)
                # custom-DVE ops and partition_broadcast mishandle source APs
                # with a non-zero partition base: stage the denominator row
                # at partition 0 with a plain copy first.
                den = spool.tile([1, 1024], F32, name="den", tag="den")
                nc.vector.tensor_copy(den[:], pso[64:65, :])
                rc = spool.tile([1, 1024], F32, name="rc", tag="rc")
                nc.vector.reciprocal_approx_fast(rc[:], den[:])
                bc = spool.tile([64, 1024], F32, name="bc", tag="bc")
                nc.gpsimd.partition_broadcast(bc[:], rc[:])
                nc.vector.tensor_mul(gh[p0:p0 + 64, g, :], pso[0:64, :], bc[:])

            for oh in range(2):
                psy = psS.tile([128, 1024], F32, name="psy", tag="psS")
                for c in range(2):
                    for g in range(4):
                        nc.tensor.matmul(
                            psy[:, c * NB:(c + 1) * NB],
                            wo_sb[:, g, oh * 128:(oh + 1) * 128],
                            gh[:, g, c * NB:(c + 1) * NB],
                            start=(g == 0), stop=(g == 3))
                y = spool.tile([128, 1024], F32, name="y", tag="y")
                nc.vector.tensor_scalar_add(y[:], psy[:], bias_sb[:, oh:oh + 1])
                nc.sync.dma_start(out[oh], y[:])

    nc.compile()
    nc.m = get_hw_module(nc.m)
    return nc


def _prep_in_maps(x, w_qkv, w_out, b_out):
    scale = float(D) ** -0.5
    x = np.asarray(x, np.float32)
    w_qkv = np.asarray(w_qkv, np.float32)
    w_out = np.asarray(w_out, np.float32)
    b_out = np.asarray(b_out, np.float32)

    x16 = x.astype(np.float16)                      # [4, 256, 2048]

    def pack_w(w):
        # w [512 out, 256 c] -> [128 (c%128), 2 (c//128), 512 out]
        return np.ascontiguousarray(
            w.T.reshape(2, 128, 512).transpose(1, 0, 2)).astype(np.float16)

    wq_p = pack_w(w_qkv[0:512] * scale)
    wk_p = pack_w(w_qkv[512:1024])
    wv_p = pack_w(w_qkv[1024:1536])
    wo_p = np.ascontiguousarray(
        w_out.T.reshape(4, 128, 256).transpose(1, 0, 2)).astype(np.float16)
    bias2 = np.ascontiguousarray(b_out.reshape(2, 128).T)

    in_maps = []
    for j in range(NCORES):
        b, q0 = j // 2, (j % 2) * QW
        xb = np.ascontiguousarray(
            x16[b].reshape(2, 128, L).transpose(1, 0, 2))   # [128, 2, L]
        xq = np.ascontiguousarray(xb[:, :, q0:q0 + QW])
        in_maps.append({"xb": xb, "xq": xq, "wq": wq_p, "wk": wk_p,
                        "wv": wv_p, "wo": wo_p, "bias2": bias2})
    return in_maps


def _run(inputs, trace=False):
    if "nc" not in _CACHE:
        _CACHE["nc"] = _build()
    nc = _CACHE["nc"]
    in_maps = _prep_in_maps(**inputs)
    res = bass_utils.run_bass_kernel_spmd(
        nc, in_maps, core_ids=list(range(NCORES)), trace=trace)
    y = np.empty((B, C, L), np.float32)
    for j in range(NCORES):
        b, q0 = j // 2, (j % 2) * QW
        o = res.results[j]["out"]                   # [2, 128, QW]
        y[b, 0:128, q0:q0 + QW] = o[0]
        y[b, 128:256, q0:q0 + QW] = o[1]
    return y, res


def kernel(x, w_qkv, w_out, b_out):
    y, _ = _run(dict(x=x, w_qkv=w_qkv, w_out=w_out, b_out=b_out), trace=False)
    return y
